# revision 22
# baseline (speedup 1.0000x reference)
"""Trainium2 Bass kernel for block-adapter Linear (nn_Linear_20847771255232).

Math:
    y = x @ W_base^T + b_base + s * adapter(x)
where the block-structured adapter folds into an effective weight:
    W_eff[j*64+e, i*64+d] = W_base[...] + s * U[d, i, j] * V[j, d, e]
(pure elementwise construction), so the whole problem is ONE
4096x4096x4096 GEMM with an on-device-constructed bf16 weight.

Sharding (8 cores): 4-way data parallel over tokens (m) x 2-way tensor
parallel over out features (o).

v2 design (vs v1 baseline at ~507us):
  - All inputs are re-laid-out HOST-side to k-major (pure relayout, no
    arithmetic): x^T [K, M_C], W^T pre-chunked [NOC, 128p, S*O5], U/V in
    the exact SBUF layouts the adapter needs. This removes ALL on-device
    transposes (v1 spent ~117us of DMA-queue time on xbar transposes and
    serialized stage->transpose->adapter chains).
  - SWDGE cast-DMAs (f32->bf16) land data directly in matmul layout with
    fully contiguous per-partition descriptors.
  - Adapter is DVE-only: one broadcast-AP tensor_mul (U bcast over e, V
    bcast over ki) + one tensor_add per ki-chunk. No PE matmuls (v1 spent
    38us of PE on broadcast matmuls).
  - GEMM uses N=512 matmuls (one PSUM bank per (mt, oc4)); bias is fused
    into the PSUM->SBUF drain via scalar_tensor_tensor against a
    PE-replicated bias tile, so no K=1 seeding matmuls in the hot loop.
  - Phase 1 (first 512 output cols) runs ki-outer across all 8 m-tiles
    (8 PSUM banks) so the PE chases the streaming x quads; phases 2-4 run
    mt-outer with W chunks double-buffered and the next chunk's adapter
    applied on DVE during the previous sweep.
"""

import numpy as np

BLOCK = 64
M_TOT, K_TOT, O_TOT = 4096, 4096, 4096
GRID_M, GRID_O = 4, 2
M_C, O_C = M_TOT // GRID_M, O_TOT // GRID_O  # 1024, 2048
S = K_TOT // 128          # 32 k-stripes
MT = M_C // 128           # 8 m-tiles
O5 = 512                  # matmul free dim = one PSUM bank of f32
NOC = O_C // O5           # 4 weight chunks
NBO = O_C // BLOCK        # 32 output blocks per core

_CACHE = {}


def build_program(num_devices=8, debug=False):
    import concourse.bacc as bacc
    import concourse.bass as bass
    import concourse.mybir as mybir
    import concourse.tile as tile

    f32 = mybir.dt.float32
    bf16 = mybir.dt.bfloat16
    mult = mybir.AluOpType.mult
    addop = mybir.AluOpType.add

    nc = bacc.Bacc(
        "TRN2",
        target_bir_lowering=False,
        debug=debug,
        num_devices=num_devices,
    )
    xt_d = nc.dram_tensor("xt", [K_TOT, M_C], f32, kind="ExternalInput").ap()
    w_d = nc.dram_tensor("wc", [NOC, 128, S * O5], f32, kind="ExternalInput").ap()
    b_d = nc.dram_tensor("bb", [O_C], f32, kind="ExternalInput").ap()
    u_d = nc.dram_tensor("uc", [128, S * NBO], f32, kind="ExternalInput").ap()
    v_d = nc.dram_tensor("vr", [128, O_C], f32, kind="ExternalInput").ap()
    s_d = nc.dram_tensor("ss", [1], f32, kind="ExternalInput").ap()
    y_d = nc.dram_tensor("yy", [M_C, O_C], f32, kind="ExternalOutput").ap()

    with tile.TileContext(nc) as tc:
        with (
            tc.tile_pool(name="const", bufs=1) as cpool,
            tc.tile_pool(name="xt", bufs=1) as xtpool,
            tc.tile_pool(name="wt", bufs=3) as wtpool,
            tc.tile_pool(name="wst", bufs=2) as wstpool,
            tc.tile_pool(name="adap", bufs=1) as apool,
            tc.tile_pool(name="outp", bufs=4) as opool,
            tc.tile_pool(name="wf8", bufs=2) as wf8pool,
            tc.tile_pool(name="ps", bufs=8, space=bass.MemorySpace.PSUM) as pspool,
        ):
            # ---------------- tiny constants ----------------
            ucols = cpool.tile([128, S * NBO], bf16)
            nc.gpsimd.dma_start(ucols[:], u_d)
            v_bf = cpool.tile([128, O_C], bf16)
            nc.gpsimd.dma_start(v_bf[:], v_d)
            s_sb = cpool.tile([1, 1], f32)
            nc.gpsimd.dma_start(s_sb[:], s_d[None, :])

            # broadcast s = S[0] to a [128,1] column via K=1 matmul
            ones_f32 = cpool.tile([1, 128], f32)
            nc.vector.memset(ones_f32[:], 1.0)
            s_ps = pspool.tile([128, O5], f32, tag="ps")
            nc.tensor.matmul(s_ps[:, 0:1], ones_f32[:], s_sb[:], start=True, stop=True)
            s_col = cpool.tile([128, 1], f32)
            nc.vector.tensor_copy(s_col[:], s_ps[:, 0:1])
            # W path is carried x64 (so the fp8 stripes' e4m3 cast of W
            # stays in the normal range); un-scaled by 1/64 in the drain.
            nc.vector.tensor_scalar_mul(s_col[:], s_col[:], 64.0)
            nc.vector.tensor_scalar_mul(ucols[:], ucols[:], s_col[:])

            u3 = ucols[:].rearrange("p (k j) -> p k j", j=NBO)

            def adapter(c, wt, q):  # ki chunk [4q, 4q+4)
                ad = apool.tile([128, 4, O5], bf16, tag="ad", name=f"adp_{c}_{q}")
                ub = (
                    u3[:, q * 4 : (q + 1) * 4, c * 8 : (c + 1) * 8]
                    .rearrange("p k (j one) -> p k j one", one=1)
                    .broadcast_to([128, 4, 8, BLOCK])
                )
                vb = (
                    v_bf[:, c * O5 : (c + 1) * O5]
                    .rearrange("p (one j e) -> p one j e", one=1, j=8)
                    .broadcast_to([128, 4, 8, BLOCK])
                )
                nc.vector.tensor_mul(
                    ad[:].rearrange("p k (j e) -> p k j e", e=BLOCK), ub, vb
                )
                nc.vector.scalar_tensor_tensor(
                    wt[:, q * 4 : (q + 1) * 4, :],
                    wt[:, q * 4 : (q + 1) * 4, :],
                    64.0,
                    ad[:],
                    op0=mult,
                    op1=addop,
                )

            # bias replicated to 128 partitions via a broadcast-source
            # cast DMA (reads the same DRAM range once per partition)
            b_rep = cpool.tile([128, O_C], bf16)
            nc.gpsimd.dma_start(
                b_rep[:], b_d[None, :].broadcast_to([128, O_C])
            )

            # ------------- big loads: quartered W + chained x -------------
            # One SWDGE cast-DMA only sustains ~150-200 GB/s, so W chunks
            # load as 4 concurrent 1MB quarters (adapter adds then also
            # start per-quarter instead of per-chunk). Guard copies keep
            # ~3 transfers in flight in consumption order (real WAW deps:
            # read a probe of a landed transfer, write a probe inside the
            # next transfer's dest).
            wts = []
            xT = xtpool.tile([128, S, M_C], bf16)
            w0 = wtpool.tile([128, S, O5], bf16, tag="wt")
            w1 = wtpool.tile([128, S, O5], bf16, tag="wt")

            def wq(wt, c, h):
                nc.gpsimd.dma_start(
                    wt[:, h * 8 : (h + 1) * 8, :].rearrange("p k o -> p (k o)"),
                    w_d[c][:, h * 8 * O5 : (h + 1) * 8 * O5],
                )

            def xq(q):
                nc.gpsimd.dma_start(
                    xT[:, 4 * q : 4 * q + 4, :],
                    xt_d[q * 512 : (q + 1) * 512, :].rearrange(
                        "(kk p) m -> p kk m", p=128
                    ),
                )

            xg = lambda q: xT[0:1, 4 * q, 0:1]
            wp = lambda wt, h: wt[0:1, 8 * h, 0:1]

            def guard(rd, wr):
                nc.gpsimd.tensor_copy(wr, rd)

            wq(w0, 0, 0)
            xq(0)
            wq(w0, 0, 1)
            guard(wp(w0, 0), wp(w0, 2)); wq(w0, 0, 2)
            guard(xg(0), xg(1)); xq(1)
            guard(wp(w0, 1), wp(w0, 3)); wq(w0, 0, 3)
            guard(wp(w0, 2), xg(2)); xq(2)
            guard(wp(w0, 3), xg(3)); xq(3)
            guard(xg(1), xg(4)); xq(4)
            guard(xg(2), wp(w1, 0)); wq(w1, 1, 0)
            guard(xg(3), xg(5)); xq(5)
            guard(xg(4), wp(w1, 1)); wq(w1, 1, 1)
            guard(xg(5), xg(6)); xq(6)
            guard(wp(w1, 0), xg(7)); xq(7)
            guard(xg(6), wp(w1, 2)); wq(w1, 1, 2)
            guard(wp(w1, 1), wp(w1, 3)); wq(w1, 1, 3)

            w2 = wtpool.tile([128, S, O5], bf16, tag="wt")
            guard(xg(7), wp(w2, 0)); wq(w2, 2, 0)
            guard(wp(w1, 2), wp(w2, 1)); wq(w2, 2, 1)
            guard(wp(w1, 3), wp(w2, 2)); wq(w2, 2, 2)
            guard(wp(w2, 0), wp(w2, 3)); wq(w2, 2, 3)

            wts.append(w0)
            wts.append(w1)

            # fp8 hybrid: stripes [KB, 32) run as e4m3 DoubleRow pair-MMs
            KB, NP = 20, 6
            f8 = mybir.dt.float8e4
            xF8 = cpool.tile([128, NP, 2, M_C], f8)

            def wf8_cast(c, c_wt):
                wf = wf8pool.tile([128, NP, 2, O5], f8, tag="wf8", name=f"wf8_{c}")
                for kp in range(NP):
                    nc.vector.tensor_copy(
                        wf[:, kp, :, :], c_wt[:, KB + 2 * kp : KB + 2 * kp + 2, :]
                    )
                return wf

            wfs = []
            for q in range(8):
                adapter(0, w0, q)
            wfs.append(wf8_cast(0, w0))
            for kp in range(NP):
                nc.scalar.copy(
                    xF8[:, kp, :, :], xT[:, KB + 2 * kp : KB + 2 * kp + 2, :]
                )
            for q in range(8):
                adapter(1, w1, q)
            wfs.append(wf8_cast(1, w1))

            def load_chunk(c, wt):
                for h in range(4):
                    wq(wt, c, h)
                for q in range(8):
                    adapter(c, wt, q)
                wfs.append(wf8_cast(c, wt))

            # ---------------- adapter: wt[p, ki, j*64+e] += s*U*V ----------------
            # ---------------- drain helper (fused bias add) ----------------
            def drain(ps, mt, c4):
                osb = opool.tile([128, O5], f32, tag="o")
                nc.vector.scalar_tensor_tensor(
                    osb[:],
                    ps[:],
                    0.015625,
                    b_rep[:, c4 * O5 : (c4 + 1) * O5],
                    op0=mult,
                    op1=addop,
                )
                eng = nc.sync if mt % 2 == 0 else nc.scalar
                eng.dma_start(
                    y_d[mt * 128 : (mt + 1) * 128, c4 * O5 : (c4 + 1) * O5], osb[:]
                )

            # ---------------- phase 1: oc4=0, ki-outer (chases x quads) --------
            ps1 = [pspool.tile([128, O5], f32, tag="ps", name=f"ps1_{i}") for i in range(MT)]
            for ki in range(KB):
                for mt in range(MT):
                    nc.tensor.matmul(
                        ps1[mt][:],
                        xT[:, ki, mt * 128 : (mt + 1) * 128],
                        w0[:, ki, :],
                        start=(ki == 0),
                        stop=False,
                    )
            for kp in range(NP):
                for mt in range(MT):
                    nc.tensor.matmul(
                        ps1[mt][:],
                        xF8[:, kp, :, mt * 128 : (mt + 1) * 128],
                        wfs[0][:, kp, :, :],
                        start=False,
                        stop=(kp == NP - 1),
                        perf_mode=mybir.MatmulPerfMode.DoubleRow,
                    )
            for mt in range(MT):
                drain(ps1[mt], mt, 0)
            # W2 was prefetched into the third buffer during phase 1;
            # its adapter + fp8 cast run on DVE during the oc1 sweep
            for q in range(8):
                adapter(2, w2, q)
            wts.append(w2)
            wfs.append(wf8_cast(2, w2))

            # ---------------- phases 2-4: mt-outer, W double-buffered ----------
            for c4 in range(1, NOC):
                for mt in range(MT):
                    ps = pspool.tile([128, O5], f32, tag="ps")
                    for ki in range(KB):
                        nc.tensor.matmul(
                            ps[:],
                            xT[:, ki, mt * 128 : (mt + 1) * 128],
                            wts[c4][:, ki, :],
                            start=(ki == 0),
                            stop=False,
                        )
                    for kp in range(NP):
                        nc.tensor.matmul(
                            ps[:],
                            xF8[:, kp, :, mt * 128 : (mt + 1) * 128],
                            wfs[c4][:, kp, :, :],
                            start=False,
                            stop=(kp == NP - 1),
                            perf_mode=mybir.MatmulPerfMode.DoubleRow,
                        )
                    drain(ps, mt, c4)
                    if mt == 1 and c4 == 1:
                        wn = wtpool.tile([128, S, O5], bf16, tag="wt")
                        load_chunk(3, wn)
                        wts.append(wn)

    nc.compile()
    return nc


def _get_program():
    key = "full"
    if key not in _CACHE:
        _CACHE[key] = build_program()
    return _CACHE[key]


def _prep_in_maps(x, W_base, b_base, U, V, S):
    """Host-side sharding + pure relayout (no arithmetic on values)."""
    B, N, DIN = x.shape
    xf = np.ascontiguousarray(x.reshape(B * N, DIN))

    ns = K_TOT // 128  # stripe count (module-level S is shadowed by scale input)
    # per-oc tensors (shared by the 4 data-parallel cores in each column)
    w_oc, b_oc, u_oc, v_oc = [], [], [], []
    for oc in range(GRID_O):
        WT = W_base[oc * O_C : (oc + 1) * O_C, :].T  # [K, O_C] view
        wc = np.ascontiguousarray(
            WT.reshape(ns, 128, NOC, O5).transpose(2, 1, 0, 3)
        ).reshape(NOC, 128, ns * O5)
        w_oc.append(wc)
        b_oc.append(np.ascontiguousarray(b_base[oc * O_C : (oc + 1) * O_C]))
        Uj = U[:, :, oc * NBO : (oc + 1) * NBO]  # [64 d, 64 i, NBO j]
        uc = np.ascontiguousarray(
            Uj.transpose(1, 0, 2).reshape(ns, 2, BLOCK, NBO).transpose(1, 2, 0, 3)
        ).reshape(128, ns * NBO)
        u_oc.append(uc)
        Vj = V[oc * NBO : (oc + 1) * NBO]  # [NBO j, 64 d, 64 e]
        vt = Vj.transpose(1, 0, 2).reshape(BLOCK, O_C)  # [d, j*64+e]
        v_oc.append(np.ascontiguousarray(np.concatenate([vt, vt], axis=0)))

    xt_mc = [
        np.ascontiguousarray(xf[mc * M_C : (mc + 1) * M_C, :].T)
        for mc in range(GRID_M)
    ]

    in_maps = []
    for c in range(8):
        mc, oc = divmod(c, GRID_O)
        in_maps.append(
            {
                "xt": xt_mc[mc],
                "wc": w_oc[oc],
                "bb": b_oc[oc],
                "uc": u_oc[oc],
                "vr": v_oc[oc],
                "ss": np.ascontiguousarray(S),
            }
        )
    return in_maps


def kernel(x, W_base, b_base, U, V, S):
    from concourse import bass_utils

    x = np.asarray(x, dtype=np.float32)
    W_base = np.asarray(W_base, dtype=np.float32)
    b_base = np.asarray(b_base, dtype=np.float32)
    U = np.asarray(U, dtype=np.float32)
    V = np.asarray(V, dtype=np.float32)
    S = np.asarray(S, dtype=np.float32)

    B, N, DIN = x.shape
    nc = _get_program()
    in_maps = _prep_in_maps(x, W_base, b_base, U, V, S)
    res = bass_utils.run_bass_kernel_spmd(nc, in_maps, core_ids=list(range(8)))

    y = np.empty((M_TOT, O_TOT), dtype=np.float32)
    for c in range(8):
        mc, oc = divmod(c, GRID_O)
        y[mc * M_C : (mc + 1) * M_C, oc * O_C : (oc + 1) * O_C] = res.results[c]["yy"]
    return y.reshape(B, N, O_TOT)


# revision 23
# speedup vs baseline: 1.0454x; 1.0454x over previous
"""Trainium2 Bass kernel for block-adapter Linear (nn_Linear_20847771255232).

Math:
    y = x @ W_base^T + b_base + s * adapter(x)
where the block-structured adapter folds into an effective weight:
    W_eff[j*64+e, i*64+d] = W_base[...] + s * U[d, i, j] * V[j, d, e]
(pure elementwise construction), so the whole problem is ONE
4096x4096x4096 GEMM with an on-device-constructed bf16 weight.

Sharding (8 cores): 4-way data parallel over tokens (m) x 2-way tensor
parallel over out features (o).

v2 design (vs v1 baseline at ~507us):
  - All inputs are re-laid-out HOST-side to k-major (pure relayout, no
    arithmetic): x^T [K, M_C], W^T pre-chunked [NOC, 128p, S*O5], U/V in
    the exact SBUF layouts the adapter needs. This removes ALL on-device
    transposes (v1 spent ~117us of DMA-queue time on xbar transposes and
    serialized stage->transpose->adapter chains).
  - SWDGE cast-DMAs (f32->bf16) land data directly in matmul layout with
    fully contiguous per-partition descriptors.
  - Adapter is DVE-only: one broadcast-AP tensor_mul (U bcast over e, V
    bcast over ki) + one tensor_add per ki-chunk. No PE matmuls (v1 spent
    38us of PE on broadcast matmuls).
  - GEMM uses N=512 matmuls (one PSUM bank per (mt, oc4)); bias is fused
    into the PSUM->SBUF drain via scalar_tensor_tensor against a
    PE-replicated bias tile, so no K=1 seeding matmuls in the hot loop.
  - Phase 1 (first 512 output cols) runs ki-outer across all 8 m-tiles
    (8 PSUM banks) so the PE chases the streaming x quads; phases 2-4 run
    mt-outer with W chunks double-buffered and the next chunk's adapter
    applied on DVE during the previous sweep.
"""

import numpy as np

BLOCK = 64
M_TOT, K_TOT, O_TOT = 4096, 4096, 4096
GRID_M, GRID_O = 4, 2
M_C, O_C = M_TOT // GRID_M, O_TOT // GRID_O  # 1024, 2048
S = K_TOT // 128          # 32 k-stripes
MT = M_C // 128           # 8 m-tiles
O5 = 512                  # matmul free dim = one PSUM bank of f32
NOC = O_C // O5           # 4 weight chunks
NBO = O_C // BLOCK        # 32 output blocks per core

_CACHE = {}


def build_program(num_devices=8, debug=False):
    import concourse.bacc as bacc
    import concourse.bass as bass
    import concourse.mybir as mybir
    import concourse.tile as tile

    f32 = mybir.dt.float32
    bf16 = mybir.dt.bfloat16
    mult = mybir.AluOpType.mult
    addop = mybir.AluOpType.add

    nc = bacc.Bacc(
        "TRN2",
        target_bir_lowering=False,
        debug=debug,
        num_devices=num_devices,
    )
    xt_d = nc.dram_tensor("xt", [K_TOT, M_C], f32, kind="ExternalInput").ap()
    w_d = nc.dram_tensor("wc", [NOC, 128, S * O5], f32, kind="ExternalInput").ap()
    b_d = nc.dram_tensor("bb", [O_C], f32, kind="ExternalInput").ap()
    u_d = nc.dram_tensor("uc", [128, S * NBO], f32, kind="ExternalInput").ap()
    v_d = nc.dram_tensor("vr", [128, O_C], f32, kind="ExternalInput").ap()
    s_d = nc.dram_tensor("ss", [1], f32, kind="ExternalInput").ap()
    y_d = nc.dram_tensor("yy", [M_C, O_C], f32, kind="ExternalOutput").ap()

    with tile.TileContext(nc) as tc:
        with (
            tc.tile_pool(name="const", bufs=1) as cpool,
            tc.tile_pool(name="xt", bufs=1) as xtpool,
            tc.tile_pool(name="wt", bufs=3) as wtpool,
            tc.tile_pool(name="wst", bufs=2) as wstpool,
            tc.tile_pool(name="adap", bufs=1) as apool,
            tc.tile_pool(name="outp", bufs=2) as opool,
            tc.tile_pool(name="wf8", bufs=2) as wf8pool,
            tc.tile_pool(name="ps", bufs=8, space=bass.MemorySpace.PSUM) as pspool,
        ):
            # ---------------- tiny constants ----------------
            ucols = cpool.tile([128, S * NBO], bf16)
            nc.gpsimd.dma_start(ucols[:], u_d)
            v_bf = cpool.tile([128, O_C], bf16)
            nc.gpsimd.dma_start(v_bf[:], v_d)
            s_sb = cpool.tile([1, 1], f32)
            nc.gpsimd.dma_start(s_sb[:], s_d[None, :])
            b_sb = cpool.tile([1, O_C], bf16)
            nc.gpsimd.dma_start(b_sb[:], b_d[None, :])  # cast f32->bf16

            # broadcast s = S[0] to a [128,1] column via K=1 matmul
            ones_f32 = cpool.tile([1, 128], f32)
            nc.vector.memset(ones_f32[:], 1.0)
            s_ps = pspool.tile([128, O5], f32, tag="ps")
            nc.tensor.matmul(s_ps[:, 0:1], ones_f32[:], s_sb[:], start=True, stop=True)
            s_col = cpool.tile([128, 1], f32)
            nc.vector.tensor_copy(s_col[:], s_ps[:, 0:1])
            # W path is carried x64 (so the fp8 stripes' e4m3 cast of W
            # stays in the normal range); un-scaled by 1/64 in the drain.
            nc.vector.tensor_scalar_mul(s_col[:], s_col[:], 64.0)
            nc.vector.tensor_scalar_mul(ucols[:], ucols[:], s_col[:])

            u3 = ucols[:].rearrange("p (k j) -> p k j", j=NBO)

            def adapter(c, wt, q):  # ki chunk [4q, 4q+4)
                ad = apool.tile([128, 4, O5], bf16, tag="ad", name=f"adp_{c}_{q}")
                ub = (
                    u3[:, q * 4 : (q + 1) * 4, c * 8 : (c + 1) * 8]
                    .rearrange("p k (j one) -> p k j one", one=1)
                    .broadcast_to([128, 4, 8, BLOCK])
                )
                vb = (
                    v_bf[:, c * O5 : (c + 1) * O5]
                    .rearrange("p (one j e) -> p one j e", one=1, j=8)
                    .broadcast_to([128, 4, 8, BLOCK])
                )
                nc.vector.tensor_mul(
                    ad[:].rearrange("p k (j e) -> p k j e", e=BLOCK), ub, vb
                )
                nc.vector.scalar_tensor_tensor(
                    wt[:, q * 4 : (q + 1) * 4, :],
                    wt[:, q * 4 : (q + 1) * 4, :],
                    64.0,
                    ad[:],
                    op0=mult,
                    op1=addop,
                )

            # bias replicated to 128 partitions (early: drains must never
            # queue behind late adapter work on the DVE FIFO)
            ones_bf = cpool.tile([1, 128], bf16)
            nc.vector.tensor_copy(ones_bf[:], ones_f32[:])
            b_rep = cpool.tile([128, O_C], bf16)
            for c4 in range(NOC):
                bp = pspool.tile([128, O5], f32, tag="ps", name=f"bp_{c4}")
                nc.tensor.matmul(
                    bp[:],
                    ones_bf[:],
                    b_sb[:, c4 * O5 : (c4 + 1) * O5],
                    start=True,
                    stop=True,
                )
                nc.vector.tensor_copy(b_rep[:, c4 * O5 : (c4 + 1) * O5], bp[:])

            # ------------- big loads: quartered W + chained x -------------
            # One SWDGE cast-DMA only sustains ~150-200 GB/s, so W chunks
            # load as 4 concurrent 1MB quarters (adapter adds then also
            # start per-quarter instead of per-chunk). Guard copies keep
            # ~3 transfers in flight in consumption order (real WAW deps:
            # read a probe of a landed transfer, write a probe inside the
            # next transfer's dest).
            wts = []
            xT = xtpool.tile([128, S, M_C], bf16)
            w0 = wtpool.tile([128, S, O5], bf16, tag="wt")
            w1 = wtpool.tile([128, S, O5], bf16, tag="wt")

            def wq(wt, c, h):
                nc.gpsimd.dma_start(
                    wt[:, h * 8 : (h + 1) * 8, :].rearrange("p k o -> p (k o)"),
                    w_d[c][:, h * 8 * O5 : (h + 1) * 8 * O5],
                )

            def xq(q):
                nc.gpsimd.dma_start(
                    xT[:, 4 * q : 4 * q + 4, :],
                    xt_d[q * 512 : (q + 1) * 512, :].rearrange(
                        "(kk p) m -> p kk m", p=128
                    ),
                )

            xg = lambda q: xT[0:1, 4 * q, 0:1]
            wp = lambda wt, h: wt[0:1, 8 * h, 0:1]

            def guard(rd, wr):
                nc.gpsimd.tensor_copy(wr, rd)

            wq(w0, 0, 0)
            xq(0)
            wq(w0, 0, 1)
            guard(wp(w0, 0), wp(w0, 2)); wq(w0, 0, 2)
            guard(xg(0), xg(1)); xq(1)
            guard(wp(w0, 1), wp(w0, 3)); wq(w0, 0, 3)
            guard(wp(w0, 2), xg(2)); xq(2)
            guard(wp(w0, 3), xg(3)); xq(3)
            guard(xg(1), xg(4)); xq(4)
            guard(xg(2), wp(w1, 0)); wq(w1, 1, 0)
            guard(xg(3), xg(5)); xq(5)
            guard(xg(4), wp(w1, 1)); wq(w1, 1, 1)
            guard(xg(5), xg(6)); xq(6)
            guard(wp(w1, 0), xg(7)); xq(7)
            guard(xg(6), wp(w1, 2)); wq(w1, 1, 2)
            guard(wp(w1, 1), wp(w1, 3)); wq(w1, 1, 3)

            w2 = wtpool.tile([128, S, O5], bf16, tag="wt")
            guard(xg(7), wp(w2, 0)); wq(w2, 2, 0)
            guard(wp(w1, 2), wp(w2, 1)); wq(w2, 2, 1)
            guard(wp(w1, 3), wp(w2, 2)); wq(w2, 2, 2)
            guard(wp(w2, 0), wp(w2, 3)); wq(w2, 2, 3)

            wts.append(w0)
            wts.append(w1)

            # fp8 hybrid: stripes [KB, 32) run as e4m3 DoubleRow pair-MMs
            KB, NP = 20, 6
            f8 = mybir.dt.float8e4
            xF8 = cpool.tile([128, NP, 2, M_C], f8)

            def wf8_cast(c, c_wt):
                wf = wf8pool.tile([128, NP, 2, O5], f8, tag="wf8", name=f"wf8_{c}")
                for kp in range(NP):
                    nc.vector.tensor_copy(
                        wf[:, kp, :, :], c_wt[:, KB + 2 * kp : KB + 2 * kp + 2, :]
                    )
                return wf

            wfs = []
            for q in range(8):
                adapter(0, w0, q)
            wfs.append(wf8_cast(0, w0))
            for kp in range(NP):
                nc.scalar.copy(
                    xF8[:, kp, :, :], xT[:, KB + 2 * kp : KB + 2 * kp + 2, :]
                )
            for q in range(8):
                adapter(1, w1, q)
            wfs.append(wf8_cast(1, w1))

            def load_chunk(c, wt):
                for h in range(4):
                    wq(wt, c, h)
                for q in range(8):
                    adapter(c, wt, q)
                wfs.append(wf8_cast(c, wt))

            # ---------------- adapter: wt[p, ki, j*64+e] += s*U*V ----------------
            # ---------------- drain helper (fused bias add) ----------------
            def drain(ps, mt, c4):
                osb = opool.tile([128, O5], f32, tag="o")
                nc.vector.scalar_tensor_tensor(
                    osb[:],
                    ps[:],
                    0.015625,
                    b_rep[:, c4 * O5 : (c4 + 1) * O5],
                    op0=mult,
                    op1=addop,
                )
                eng = nc.sync if mt % 2 == 0 else nc.scalar
                eng.dma_start(
                    y_d[mt * 128 : (mt + 1) * 128, c4 * O5 : (c4 + 1) * O5], osb[:]
                )

            # ---------------- phase 1: oc4=0, ki-outer (chases x quads) --------
            ps1 = [pspool.tile([128, O5], f32, tag="ps", name=f"ps1_{i}") for i in range(MT)]
            for ki in range(KB):
                for mt in range(MT):
                    nc.tensor.matmul(
                        ps1[mt][:],
                        xT[:, ki, mt * 128 : (mt + 1) * 128],
                        w0[:, ki, :],
                        start=(ki == 0),
                        stop=False,
                    )
            for kp in range(NP):
                for mt in range(MT):
                    nc.tensor.matmul(
                        ps1[mt][:],
                        xF8[:, kp, :, mt * 128 : (mt + 1) * 128],
                        wfs[0][:, kp, :, :],
                        start=False,
                        stop=(kp == NP - 1),
                        perf_mode=mybir.MatmulPerfMode.DoubleRow,
                    )
            for mt in range(MT):
                drain(ps1[mt], mt, 0)
            # W2 was prefetched into the third buffer during phase 1;
            # its adapter + fp8 cast run on DVE during the oc1 sweep
            for q in range(8):
                adapter(2, w2, q)
            wts.append(w2)
            wfs.append(wf8_cast(2, w2))

            # ---------------- phases 2-4: mt-outer, W double-buffered ----------
            for c4 in range(1, NOC):
                for mt in range(MT):
                    ps = pspool.tile([128, O5], f32, tag="ps")
                    for ki in range(KB):
                        nc.tensor.matmul(
                            ps[:],
                            xT[:, ki, mt * 128 : (mt + 1) * 128],
                            wts[c4][:, ki, :],
                            start=(ki == 0),
                            stop=False,
                        )
                    for kp in range(NP):
                        nc.tensor.matmul(
                            ps[:],
                            xF8[:, kp, :, mt * 128 : (mt + 1) * 128],
                            wfs[c4][:, kp, :, :],
                            start=False,
                            stop=(kp == NP - 1),
                            perf_mode=mybir.MatmulPerfMode.DoubleRow,
                        )
                    drain(ps, mt, c4)
                    if mt == 1 and c4 == 1:
                        wn = wtpool.tile([128, S, O5], bf16, tag="wt")
                        load_chunk(3, wn)
                        wts.append(wn)

    nc.compile()
    return nc


def _get_program():
    key = "full"
    if key not in _CACHE:
        _CACHE[key] = build_program()
    return _CACHE[key]


def _prep_in_maps(x, W_base, b_base, U, V, S):
    """Host-side sharding + pure relayout (no arithmetic on values)."""
    B, N, DIN = x.shape
    xf = np.ascontiguousarray(x.reshape(B * N, DIN))

    ns = K_TOT // 128  # stripe count (module-level S is shadowed by scale input)
    # per-oc tensors (shared by the 4 data-parallel cores in each column)
    w_oc, b_oc, u_oc, v_oc = [], [], [], []
    for oc in range(GRID_O):
        WT = W_base[oc * O_C : (oc + 1) * O_C, :].T  # [K, O_C] view
        wc = np.ascontiguousarray(
            WT.reshape(ns, 128, NOC, O5).transpose(2, 1, 0, 3)
        ).reshape(NOC, 128, ns * O5)
        w_oc.append(wc)
        b_oc.append(np.ascontiguousarray(b_base[oc * O_C : (oc + 1) * O_C]))
        Uj = U[:, :, oc * NBO : (oc + 1) * NBO]  # [64 d, 64 i, NBO j]
        uc = np.ascontiguousarray(
            Uj.transpose(1, 0, 2).reshape(ns, 2, BLOCK, NBO).transpose(1, 2, 0, 3)
        ).reshape(128, ns * NBO)
        u_oc.append(uc)
        Vj = V[oc * NBO : (oc + 1) * NBO]  # [NBO j, 64 d, 64 e]
        vt = Vj.transpose(1, 0, 2).reshape(BLOCK, O_C)  # [d, j*64+e]
        v_oc.append(np.ascontiguousarray(np.concatenate([vt, vt], axis=0)))

    xt_mc = [
        np.ascontiguousarray(xf[mc * M_C : (mc + 1) * M_C, :].T)
        for mc in range(GRID_M)
    ]

    in_maps = []
    for c in range(8):
        mc, oc = divmod(c, GRID_O)
        in_maps.append(
            {
                "xt": xt_mc[mc],
                "wc": w_oc[oc],
                "bb": b_oc[oc],
                "uc": u_oc[oc],
                "vr": v_oc[oc],
                "ss": np.ascontiguousarray(S),
            }
        )
    return in_maps


def kernel(x, W_base, b_base, U, V, S):
    from concourse import bass_utils

    x = np.asarray(x, dtype=np.float32)
    W_base = np.asarray(W_base, dtype=np.float32)
    b_base = np.asarray(b_base, dtype=np.float32)
    U = np.asarray(U, dtype=np.float32)
    V = np.asarray(V, dtype=np.float32)
    S = np.asarray(S, dtype=np.float32)

    B, N, DIN = x.shape
    nc = _get_program()
    in_maps = _prep_in_maps(x, W_base, b_base, U, V, S)
    res = bass_utils.run_bass_kernel_spmd(nc, in_maps, core_ids=list(range(8)))

    y = np.empty((M_TOT, O_TOT), dtype=np.float32)
    for c in range(8):
        mc, oc = divmod(c, GRID_O)
        y[mc * M_C : (mc + 1) * M_C, oc * O_C : (oc + 1) * O_C] = res.results[c]["yy"]
    return y.reshape(B, N, O_TOT)


# revision 24
# speedup vs baseline: 1.2302x; 1.1767x over previous
"""Trainium2 Bass kernel for block-adapter Linear (nn_Linear_20847771255232).

Math:
    y = x @ W_base^T + b_base + s * adapter(x)
where the block-structured adapter folds into an effective weight:
    W_eff[j*64+e, i*64+d] = W_base[...] + s * U[d, i, j] * V[j, d, e]
(pure elementwise construction), so the whole problem is ONE
4096x4096x4096 GEMM with an on-device-constructed bf16 weight.

Sharding (8 cores): 4-way data parallel over tokens (m) x 2-way tensor
parallel over out features (o).

v2 design (vs v1 baseline at ~507us):
  - All inputs are re-laid-out HOST-side to k-major (pure relayout, no
    arithmetic): x^T [K, M_C], W^T pre-chunked [NOC, 128p, S*O5], U/V in
    the exact SBUF layouts the adapter needs. This removes ALL on-device
    transposes (v1 spent ~117us of DMA-queue time on xbar transposes and
    serialized stage->transpose->adapter chains).
  - SWDGE cast-DMAs (f32->bf16) land data directly in matmul layout with
    fully contiguous per-partition descriptors.
  - Adapter is DVE-only: one broadcast-AP tensor_mul (U bcast over e, V
    bcast over ki) + one tensor_add per ki-chunk. No PE matmuls (v1 spent
    38us of PE on broadcast matmuls).
  - GEMM uses N=512 matmuls (one PSUM bank per (mt, oc4)); bias is fused
    into the PSUM->SBUF drain via scalar_tensor_tensor against a
    PE-replicated bias tile, so no K=1 seeding matmuls in the hot loop.
  - Phase 1 (first 512 output cols) runs ki-outer across all 8 m-tiles
    (8 PSUM banks) so the PE chases the streaming x quads; phases 2-4 run
    mt-outer with W chunks double-buffered and the next chunk's adapter
    applied on DVE during the previous sweep.
"""

import numpy as np

BLOCK = 64
M_TOT, K_TOT, O_TOT = 4096, 4096, 4096
GRID_M, GRID_O = 4, 2
M_C, O_C = M_TOT // GRID_M, O_TOT // GRID_O  # 1024, 2048
S = K_TOT // 128          # 32 k-stripes
MT = M_C // 128           # 8 m-tiles
O5 = 512                  # matmul free dim = one PSUM bank of f32
NOC = O_C // O5           # 4 weight chunks
NBO = O_C // BLOCK        # 32 output blocks per core

_CACHE = {}


def build_program(num_devices=8, debug=False):
    import concourse.bacc as bacc
    import concourse.bass as bass
    import concourse.mybir as mybir
    import concourse.tile as tile

    f32 = mybir.dt.float32
    bf16 = mybir.dt.bfloat16
    mult = mybir.AluOpType.mult
    addop = mybir.AluOpType.add

    nc = bacc.Bacc(
        "TRN2",
        target_bir_lowering=False,
        debug=debug,
        num_devices=num_devices,
    )
    xt_d = nc.dram_tensor("xt", [K_TOT, M_C], f32, kind="ExternalInput").ap()
    w_d = nc.dram_tensor("wc", [NOC, 128, S * O5], f32, kind="ExternalInput").ap()
    b_d = nc.dram_tensor("bb", [O_C], f32, kind="ExternalInput").ap()
    u_d = nc.dram_tensor("uc", [128, S * NBO], f32, kind="ExternalInput").ap()
    v_d = nc.dram_tensor("vr", [128, O_C], f32, kind="ExternalInput").ap()
    s_d = nc.dram_tensor("ss", [1], f32, kind="ExternalInput").ap()
    y_d = nc.dram_tensor("yy", [M_C, O_C], f32, kind="ExternalOutput").ap()

    with tile.TileContext(nc) as tc:
        with (
            tc.tile_pool(name="const", bufs=1) as cpool,
            tc.tile_pool(name="xt", bufs=1) as xtpool,
            tc.tile_pool(name="wt", bufs=3) as wtpool,
            tc.tile_pool(name="wst", bufs=2) as wstpool,
            tc.tile_pool(name="adap", bufs=1) as apool,
            tc.tile_pool(name="outp", bufs=4) as opool,
            tc.tile_pool(name="wf8", bufs=2) as wf8pool,
            tc.tile_pool(name="ps", bufs=8, space=bass.MemorySpace.PSUM) as pspool,
        ):
            # ---------------- tiny constants ----------------
            ucols = cpool.tile([128, S * NBO], bf16)
            nc.gpsimd.dma_start(ucols[:], u_d)
            v_bf = cpool.tile([128, O_C], bf16)
            nc.gpsimd.dma_start(v_bf[:], v_d)
            s_sb = cpool.tile([1, 1], f32)
            nc.gpsimd.dma_start(s_sb[:], s_d[None, :])
            # b staging borrows the adapter pool's slot: it is consumed
            # by the b_rep build (~7us in) before the first ad tile needs
            # the slot, so this costs no SBUF
            b_sb = apool.tile([1, O_C], bf16, tag="ad", name="b_sb")
            nc.gpsimd.dma_start(b_sb[:], b_d[None, :])  # cast f32->bf16

            # broadcast s = S[0] to a [128,1] column via K=1 matmul
            ones_f32 = cpool.tile([1, 128], f32)
            nc.vector.memset(ones_f32[:], 1.0)
            s_ps = pspool.tile([128, O5], f32, tag="ps")
            nc.tensor.matmul(s_ps[:, 0:1], ones_f32[:], s_sb[:], start=True, stop=True)
            s_col = cpool.tile([128, 1], f32)
            nc.vector.tensor_copy(s_col[:], s_ps[:, 0:1])
            # W path is carried x64 (so the fp8 stripes' e4m3 cast of W
            # stays in the normal range); un-scaled by 1/64 in the drain.
            nc.vector.tensor_scalar_mul(s_col[:], s_col[:], 64.0)
            nc.vector.tensor_scalar_mul(ucols[:], ucols[:], s_col[:])

            u3 = ucols[:].rearrange("p (k j) -> p k j", j=NBO)

            def adapter(c, wt, q):  # ki chunk [4q, 4q+4)
                ad = apool.tile([128, 4, O5], bf16, tag="ad", name=f"adp_{c}_{q}")
                ub = (
                    u3[:, q * 4 : (q + 1) * 4, c * 8 : (c + 1) * 8]
                    .rearrange("p k (j one) -> p k j one", one=1)
                    .broadcast_to([128, 4, 8, BLOCK])
                )
                vb = (
                    v_bf[:, c * O5 : (c + 1) * O5]
                    .rearrange("p (one j e) -> p one j e", one=1, j=8)
                    .broadcast_to([128, 4, 8, BLOCK])
                )
                nc.vector.tensor_mul(
                    ad[:].rearrange("p k (j e) -> p k j e", e=BLOCK), ub, vb
                )
                nc.vector.scalar_tensor_tensor(
                    wt[:, q * 4 : (q + 1) * 4, :],
                    wt[:, q * 4 : (q + 1) * 4, :],
                    64.0,
                    ad[:],
                    op0=mult,
                    op1=addop,
                )

            # bias replicated to 128 partitions (early: drains must never
            # queue behind late adapter work on the DVE FIFO)
            ones_bf = cpool.tile([1, 128], bf16)
            nc.vector.tensor_copy(ones_bf[:], ones_f32[:])
            b_rep = cpool.tile([128, O_C], bf16)
            for c4 in range(NOC):
                bp = pspool.tile([128, O5], f32, tag="ps", name=f"bp_{c4}")
                nc.tensor.matmul(
                    bp[:],
                    ones_bf[:],
                    b_sb[:, c4 * O5 : (c4 + 1) * O5],
                    start=True,
                    stop=True,
                )
                nc.vector.tensor_copy(b_rep[:, c4 * O5 : (c4 + 1) * O5], bp[:])

            # ------------- big loads: quartered W + chained x -------------
            # One SWDGE cast-DMA only sustains ~150-200 GB/s, so W chunks
            # load as 4 concurrent 1MB quarters (adapter adds then also
            # start per-quarter instead of per-chunk). Guard copies keep
            # ~3 transfers in flight in consumption order (real WAW deps:
            # read a probe of a landed transfer, write a probe inside the
            # next transfer's dest).
            wts = []
            xT = xtpool.tile([128, S, M_C], bf16)
            w0 = wtpool.tile([128, S, O5], bf16, tag="wt")
            w1 = wtpool.tile([128, S, O5], bf16, tag="wt")

            def wq(wt, c, h):
                nc.gpsimd.dma_start(
                    wt[:, h * 8 : (h + 1) * 8, :].rearrange("p k o -> p (k o)"),
                    w_d[c][:, h * 8 * O5 : (h + 1) * 8 * O5],
                )

            def xq(q):
                nc.gpsimd.dma_start(
                    xT[:, 4 * q : 4 * q + 4, :],
                    xt_d[q * 512 : (q + 1) * 512, :].rearrange(
                        "(kk p) m -> p kk m", p=128
                    ),
                )

            xg = lambda q: xT[0:1, 4 * q, 0:1]
            wp = lambda wt, h: wt[0:1, 8 * h, 0:1]

            def guard(rd, wr):
                nc.gpsimd.tensor_copy(wr, rd)

            wq(w0, 0, 0)
            xq(0)
            wq(w0, 0, 1)
            guard(wp(w0, 0), wp(w0, 2)); wq(w0, 0, 2)
            guard(xg(0), xg(1)); xq(1)
            guard(wp(w0, 1), wp(w0, 3)); wq(w0, 0, 3)
            guard(wp(w0, 2), xg(2)); xq(2)
            guard(wp(w0, 3), xg(3)); xq(3)
            guard(xg(1), xg(4)); xq(4)
            guard(xg(2), wp(w1, 0)); wq(w1, 1, 0)
            guard(xg(3), xg(5)); xq(5)
            guard(xg(4), wp(w1, 1)); wq(w1, 1, 1)
            guard(xg(5), xg(6)); xq(6)
            guard(wp(w1, 0), xg(7)); xq(7)
            guard(xg(6), wp(w1, 2)); wq(w1, 1, 2)
            guard(wp(w1, 1), wp(w1, 3)); wq(w1, 1, 3)

            w2 = wtpool.tile([128, S, O5], bf16, tag="wt")
            guard(xg(7), wp(w2, 0)); wq(w2, 2, 0)
            guard(wp(w1, 2), wp(w2, 1)); wq(w2, 2, 1)
            guard(wp(w1, 3), wp(w2, 2)); wq(w2, 2, 2)
            guard(wp(w2, 0), wp(w2, 3)); wq(w2, 2, 3)

            wts.append(w0)
            wts.append(w1)

            # fp8 hybrid: stripes [KB, 32) run as e4m3 DoubleRow pair-MMs
            KB, NP = 20, 6
            f8 = mybir.dt.float8e4
            xF8 = cpool.tile([128, NP, 2, M_C], f8)

            def wf8_cast(c, c_wt):
                wf = wf8pool.tile([128, NP, 2, O5], f8, tag="wf8", name=f"wf8_{c}")
                for kp in range(NP):
                    nc.vector.tensor_copy(
                        wf[:, kp, :, :], c_wt[:, KB + 2 * kp : KB + 2 * kp + 2, :]
                    )
                return wf

            wfs = []
            for q in range(8):
                adapter(0, w0, q)
            wfs.append(wf8_cast(0, w0))
            for kp in range(NP):
                nc.scalar.copy(
                    xF8[:, kp, :, :], xT[:, KB + 2 * kp : KB + 2 * kp + 2, :]
                )
            for q in range(8):
                adapter(1, w1, q)
            wfs.append(wf8_cast(1, w1))

            def load_chunk(c, wt):
                for h in range(4):
                    wq(wt, c, h)
                for q in range(8):
                    adapter(c, wt, q)
                wfs.append(wf8_cast(c, wt))

            # ---------------- adapter: wt[p, ki, j*64+e] += s*U*V ----------------
            # ---------------- drain helper (fused bias add) ----------------
            def drain(ps, mt, c4):
                osb = opool.tile([128, O5], f32, tag="o")
                nc.vector.scalar_tensor_tensor(
                    osb[:],
                    ps[:],
                    0.015625,
                    b_rep[:, c4 * O5 : (c4 + 1) * O5],
                    op0=mult,
                    op1=addop,
                )
                eng = nc.sync if mt % 2 == 0 else nc.scalar
                eng.dma_start(
                    y_d[mt * 128 : (mt + 1) * 128, c4 * O5 : (c4 + 1) * O5], osb[:]
                )

            # ---------------- phase 1: oc4=0, ki-outer (chases x quads) --------
            ps1 = [pspool.tile([128, O5], f32, tag="ps", name=f"ps1_{i}") for i in range(MT)]
            for ki in range(KB):
                for mt in range(MT):
                    nc.tensor.matmul(
                        ps1[mt][:],
                        xT[:, ki, mt * 128 : (mt + 1) * 128],
                        w0[:, ki, :],
                        start=(ki == 0),
                        stop=False,
                    )
            for kp in range(NP):
                for mt in range(MT):
                    nc.tensor.matmul(
                        ps1[mt][:],
                        xF8[:, kp, :, mt * 128 : (mt + 1) * 128],
                        wfs[0][:, kp, :, :],
                        start=False,
                        stop=(kp == NP - 1),
                        perf_mode=mybir.MatmulPerfMode.DoubleRow,
                    )
            for mt in range(MT):
                drain(ps1[mt], mt, 0)
            # W2 was prefetched into the third buffer during phase 1;
            # its adapter + fp8 cast run on DVE during the oc1 sweep
            for q in range(8):
                adapter(2, w2, q)
            wts.append(w2)
            wfs.append(wf8_cast(2, w2))

            # ---------------- phases 2-4: mt-outer, W double-buffered ----------
            for c4 in range(1, NOC):
                for mt in range(MT):
                    ps = pspool.tile([128, O5], f32, tag="ps")
                    for ki in range(KB):
                        nc.tensor.matmul(
                            ps[:],
                            xT[:, ki, mt * 128 : (mt + 1) * 128],
                            wts[c4][:, ki, :],
                            start=(ki == 0),
                            stop=False,
                        )
                    for kp in range(NP):
                        nc.tensor.matmul(
                            ps[:],
                            xF8[:, kp, :, mt * 128 : (mt + 1) * 128],
                            wfs[c4][:, kp, :, :],
                            start=False,
                            stop=(kp == NP - 1),
                            perf_mode=mybir.MatmulPerfMode.DoubleRow,
                        )
                    drain(ps, mt, c4)
                    if mt == 1 and c4 == 1:
                        wn = wtpool.tile([128, S, O5], bf16, tag="wt")
                        load_chunk(3, wn)
                        wts.append(wn)

    nc.compile()
    return nc


def _get_program():
    key = "full"
    if key not in _CACHE:
        _CACHE[key] = build_program()
    return _CACHE[key]


def _prep_in_maps(x, W_base, b_base, U, V, S):
    """Host-side sharding + pure relayout (no arithmetic on values)."""
    B, N, DIN = x.shape
    xf = np.ascontiguousarray(x.reshape(B * N, DIN))

    ns = K_TOT // 128  # stripe count (module-level S is shadowed by scale input)
    # per-oc tensors (shared by the 4 data-parallel cores in each column)
    w_oc, b_oc, u_oc, v_oc = [], [], [], []
    for oc in range(GRID_O):
        WT = W_base[oc * O_C : (oc + 1) * O_C, :].T  # [K, O_C] view
        wc = np.ascontiguousarray(
            WT.reshape(ns, 128, NOC, O5).transpose(2, 1, 0, 3)
        ).reshape(NOC, 128, ns * O5)
        w_oc.append(wc)
        b_oc.append(np.ascontiguousarray(b_base[oc * O_C : (oc + 1) * O_C]))
        Uj = U[:, :, oc * NBO : (oc + 1) * NBO]  # [64 d, 64 i, NBO j]
        uc = np.ascontiguousarray(
            Uj.transpose(1, 0, 2).reshape(ns, 2, BLOCK, NBO).transpose(1, 2, 0, 3)
        ).reshape(128, ns * NBO)
        u_oc.append(uc)
        Vj = V[oc * NBO : (oc + 1) * NBO]  # [NBO j, 64 d, 64 e]
        vt = Vj.transpose(1, 0, 2).reshape(BLOCK, O_C)  # [d, j*64+e]
        v_oc.append(np.ascontiguousarray(np.concatenate([vt, vt], axis=0)))

    xt_mc = [
        np.ascontiguousarray(xf[mc * M_C : (mc + 1) * M_C, :].T)
        for mc in range(GRID_M)
    ]

    in_maps = []
    for c in range(8):
        mc, oc = divmod(c, GRID_O)
        in_maps.append(
            {
                "xt": xt_mc[mc],
                "wc": w_oc[oc],
                "bb": b_oc[oc],
                "uc": u_oc[oc],
                "vr": v_oc[oc],
                "ss": np.ascontiguousarray(S),
            }
        )
    return in_maps


def kernel(x, W_base, b_base, U, V, S):
    from concourse import bass_utils

    x = np.asarray(x, dtype=np.float32)
    W_base = np.asarray(W_base, dtype=np.float32)
    b_base = np.asarray(b_base, dtype=np.float32)
    U = np.asarray(U, dtype=np.float32)
    V = np.asarray(V, dtype=np.float32)
    S = np.asarray(S, dtype=np.float32)

    B, N, DIN = x.shape
    nc = _get_program()
    in_maps = _prep_in_maps(x, W_base, b_base, U, V, S)
    res = bass_utils.run_bass_kernel_spmd(nc, in_maps, core_ids=list(range(8)))

    y = np.empty((M_TOT, O_TOT), dtype=np.float32)
    for c in range(8):
        mc, oc = divmod(c, GRID_O)
        y[mc * M_C : (mc + 1) * M_C, oc * O_C : (oc + 1) * O_C] = res.results[c]["yy"]
    return y.reshape(B, N, O_TOT)
